# revision 49
# baseline (speedup 1.0000x reference)
"""Trainium2 Bass kernel for the MLPConstructor2 adjacency problem.

Computes, per batch b (one NeuronCore each, 8-way data parallel over B):
    adj[i, j] = tanh(relu(x1_i @ w1 + x2_j @ w2 + b))
for the four (spatial/temporal) quadrants of a (2560, 2560) output.

v8 design (ACT-bound, ~46us ScalarE floor):
- Output is stored as fp16 (tolerance is 2e-2; fp16 adds ~1e-3), halving
  the HBM store traffic to 13.1 MB/core. tanh(relu(x)) == relu(tanh(x)),
  so relu runs first (fused on VectorE) and tanh's fp16 result stores
  directly.
- Column vectors are built fully ON-CHIP (the DRAM scratch round-trip
  costs ~6us/hop in SDMA completion latency): VectorE 32x32-block
  transposes put x^T and w^T on partitions, then 4 rank-32 TensorE
  matmuls per half compute s2 = w2 . x_j AND replicate it across all
  128 partitions into PSUM, with the rhs free dims restrided so PSUM
  column order equals j; one VectorE copy lands each half in SBUF fp16.
- Row scalars: one fused tensor_tensor_reduce per quadrant from the
  same (p t) staging (slot t = row p*nt + t), quadrant biases folded in.
- Output blocks are row-strided to match the (p t) slot layout: block t
  covers rows {16p+t} u {16p+t+8}; per block 4 fused VectorE
  tensor_scalar (add row scalar, max 0) fill an fp16 tmp [128, 5120],
  ScalarE runs ONE tanh (fp16 in/out), one 1.31 MB store writes the 256
  strided rows. First and last blocks are split for a faster ramp/tail.
- All weights+biases arrive pre-packed in one "wpack" input (host-side
  concat), one broadcast DMA. Dummy tanh + 1x1 matmul at t=0 pull the
  ACT table load and PE cold-start off the critical path.
"""

import numpy as np
from contextlib import ExitStack

import concourse.bacc as bacc
import concourse.mybir as mybir
import concourse.tile as tile
from concourse.bass_utils import run_bass_kernel_spmd

B, N, T, D = 8, 2048, 512, 32
W = N + T                     # 2560
NT, TT = N // 128, T // 128   # 16, 4 stat slots per partition
F32 = mybir.dt.float32
F16 = mybir.dt.float16
QUADS = ("ss", "st", "ts", "tt")


def _emit(tc, sp, tm, wp_in, adj):
    nc = tc.nc
    AF = mybir.ActivationFunctionType
    OP = mybir.AluOpType
    with ExitStack() as ctx:
        ctx.enter_context(nc.allow_low_precision(
            reason="fp16 intermediates; tolerance is 2e-2, fp16 adds ~1e-3"
        ))
        const = ctx.enter_context(tc.tile_pool(name="const", bufs=1))
        tmpp = ctx.enter_context(tc.tile_pool(name="tmpp", bufs=2))
        outp = ctx.enter_context(tc.tile_pool(name="outp", bufs=3))
        psum = ctx.enter_context(tc.tile_pool(name="psum", bufs=1, space="PSUM"))

        # ---- one broadcast load for all weights + biases ------------------
        # wpack = [w_ss | w_st | w_ts | w_tt | b_ss b_st b_ts b_tt] (260 f32)
        wp = const.tile([128, 260], F32)
        nc.scalar.dma_start(wp[:], wp_in.unsqueeze(0).broadcast_to((128, 260)))

        def w_row(q):  # first half of w_q: row-side weights
            return wp[:, 64 * q : 64 * q + D]

        def b_q(q):
            return wp[:, 256 + q : 257 + q]

        # ---- warmups: ACT table load + PE cold-start at t=0 ---------------
        warm = const.tile([128, 1], F32)
        nc.vector.memset(warm[:], 0.0)
        nc.scalar.activation(warm[:], warm[:], AF.Tanh)

        # ---- stage inputs, (p t) layout: row p*nt+t at [p, t*D:(t+1)*D] ---
        x_tm = const.tile([128, TT * D], F32)
        nc.sync.dma_start(x_tm[:], tm.rearrange("(p t) d -> p t d", p=128))
        x_sp = const.tile([128, NT * D], F32)
        nc.sync.dma_start(x_sp[:], sp.rearrange("(p t) d -> p t d", p=128))
        # row-rotated copies (partition p <- row (64+p)*nt+t, p<64): the PE
        # column builder needs every 32-row group at partition base 0/32 --
        # base 96 (and explicit tile_position) is rejected/crashes.
        x_tm_sh = const.tile([64, TT * D], F32)
        nc.scalar.dma_start(
            x_tm_sh[:], tm.rearrange("(p t) d -> p t d", p=128)[64:128]
        )
        x_sp_sh = const.tile([64, NT * D], F32)
        nc.scalar.dma_start(
            x_sp_sh[:], sp.rearrange("(p t) d -> p t d", p=128)[64:128]
        )

        # ---- transposed operands for the PE column builder ----------------
        # wT[32a+d, j] = w_col_q(d): 32x32-block transpose of the (identical
        # across partitions) wp columns; then materialized to [32, 128] rows.
        wT = const.tile([128, 4 * D], F32)
        for q in range(4):
            nc.vector.transpose(
                wT[:, 32 * q : 32 * (q + 1)], wp[:, 64 * q + D : 64 * q + 2 * D]
            )
        ones = const.tile([128, 128], F32)
        nc.vector.memset(ones[:], 1.0)
        wmat = const.tile([128, 4 * 128], F32)   # q at [128q : 128(q+1)]
        for q in range(4):
            nc.vector.tensor_scalar(
                wmat[:, 128 * q : 128 * (q + 1)], ones[:],
                wT[:, 32 * q : 32 * q + 1], None, OP.mult,
            )
        # xT[32a+d, 32t+p'] = x[(32a+p')*nt + t, d]; lo = groups 0,1 and
        # hi = groups 2,3 (from the rotated staging), all at bases 0/32.
        xT_sp = const.tile([64, NT * D], F32, name="xT_sp", tag="xT_sp")
        nc.vector.transpose(xT_sp[:], x_sp[0:64, :])
        xT_sph = const.tile([64, NT * D], F32, name="xT_sph", tag="xT_sph")
        nc.vector.transpose(xT_sph[:], x_sp_sh[:])
        xT_tm = const.tile([64, TT * D], F32, name="xT_tm", tag="xT_tm")
        nc.vector.transpose(xT_tm[:], x_tm[0:64, :])
        xT_tmh = const.tile([64, TT * D], F32, name="xT_tmh", tag="xT_tmh")
        nc.vector.transpose(xT_tmh[:], x_tm_sh[:])

        # col[q, j] = w_col . x_j + (bias folded into row stats):
        # per 32-row group a, psum[:, ca + p'*nt + t] = sum_d wmat[d, :] *
        # xT[32a+d, (p', t)] -- column order == j, so one contiguous copy.
        # psum column f = 512a + 32t + p'; output col j = (32a+p')*nt + t,
        # so the psum->SBUF copy un-interleaves via its (strided) input AP.
        def col_half(q_sp, q_tm, name):
            col = const.tile([128, W], F16, name=f"col_{name}", tag=f"col_{name}")
            psn = psum.tile([128, N], F32, name=f"psn_{name}", tag="psn")
            for a in range(4):
                xs = (xT_sp, xT_sph)[a // 2]
                b = 32 * (a % 2)
                nc.tensor.matmul(
                    psn[:, 512 * a : 512 * (a + 1)],
                    wmat[b : b + 32, 128 * q_sp : 128 * (q_sp + 1)],
                    xs[b : b + 32, :],
                )
            for c in range(2):   # copy group-pairs as their matmuls finish
                nc.vector.tensor_copy(
                    col[:, 1024 * c : 1024 * (c + 1)].rearrange(
                        "q (a p t) -> q a p t", a=2, t=NT
                    ),
                    psn[:, 1024 * c : 1024 * (c + 1)].rearrange(
                        "q (a t p) -> q a p t", a=2, t=NT
                    ),
                )
            # T-part groups each get their own PSUM bank: matmul outputs
            # packed at sub-bank offsets crash the device at runtime.
            pst = psum.tile([128, N], F32, name=f"pst_{name}", tag="pst")
            for a in range(4):
                xs = (xT_tm, xT_tmh)[a // 2]
                b = 32 * (a % 2)
                nc.tensor.matmul(
                    pst[:, 512 * a : 512 * a + 128],
                    wmat[b : b + 32, 128 * q_tm : 128 * (q_tm + 1)],
                    xs[b : b + 32, :],
                )
            nc.vector.tensor_copy(
                col[:, N:W].rearrange("q (a p t) -> q a p t", a=4, t=TT),
                pst[:].rearrange("q (a t p) -> q a t p", a=4, t=NT)[:, :, 0:TT, :]
                .rearrange("q a t p -> q a p t"),
            )
            return col

        # row stats (slot t = row p*nt + t), quadrant biases folded in.
        def rstat(x, nt, q, dst, name):
            prod = const.tile([128, nt * D], F32, name=f"prod_{name}", tag="prod")
            x3 = x[:].rearrange("p (t d) -> p t d", t=nt)
            p3 = prod[:].rearrange("p (t d) -> p t d", t=nt)
            w3 = w_row(q).unsqueeze(1).broadcast_to((128, nt, D))
            nc.vector.tensor_tensor(p3, x3, w3, OP.mult)
            nc.vector.tensor_reduce(dst, p3, axis=mybir.AxisListType.X, op=OP.add)
            nc.vector.tensor_scalar_add(dst, dst, b_q(q))

        col_sp = col_half(0, 1, "sp")    # w_ss2, w_st2
        r_sp = const.tile([128, 2 * NT], F32)
        rstat(x_sp, NT, 0, r_sp[:, 0:NT], "r_ss")
        rstat(x_sp, NT, 1, r_sp[:, NT:], "r_st")

        # ---- main loop: strided 256-row blocks -----------------------------
        # spatial block t (t=0..7): rows {16p+t} (h=0) and {16p+t+8} (h=1)
        # temporal block t (t=0..1): rows 2048 + {4p+t} and 2048 + {4p+t+2}
        def block(k, t, base, nt, col, rst, hs, split=False):
            tmp = tmpp.tile([128, 2 * W], F16, name=f"tmp{k}", tag="tmp")
            ot = outp.tile([128, 2 * W], F16, name=f"ot{k}", tag="ot")
            quad = adj[base : base + 128 * nt, :]
            for h in range(2):
                o = h * W
                s = t + h * hs
                rn = rst[:, s : s + 1]
                rt = rst[:, nt + s : nt + s + 1]
                nc.vector.tensor_scalar(
                    tmp[:, o + N : o + W], col[:, N:W], rt, 0.0, OP.add, OP.max
                )
                nc.vector.tensor_scalar(
                    tmp[:, o : o + N], col[:, 0:N], rn, 0.0, OP.add, OP.max
                )
                if split:
                    nc.scalar.activation(ot[:, o : o + W], tmp[:, o : o + W], AF.Tanh)
                    nc.sync.dma_start(
                        quad.rearrange("(p r) w -> p r w", p=128)[:, s : s + 1, :],
                        ot[:, o : o + W].rearrange("p (r w) -> p r w", r=1),
                    )
            if not split:
                nc.scalar.activation(ot[:], tmp[:], AF.Tanh)
                # partition p -> rows base + nt*p + t and base + nt*p + t + hs
                nc.sync.dma_start(
                    quad.rearrange("(p g r) w -> p g r w", p=128, g=2)[
                        :, :, t : t + 1, :
                    ],
                    ot[:].rearrange("p (g w) -> p g w", g=2).unsqueeze(2),
                )

        block(0, 0, 0, NT, col_sp, r_sp, NT // 2, split=True)

        # temporal-column stats + blocks emitted after block 0 so their
        # on-chip col build (PE + VectorE copies) stays off block 0's path.
        col_tm = col_half(2, 3, "tm")    # w_ts2, w_tt2
        r_tm = const.tile([128, 2 * TT], F32)
        rstat(x_tm, TT, 2, r_tm[:, 0:TT], "r_ts")
        rstat(x_tm, TT, 3, r_tm[:, TT:], "r_tt")

        for t in range(1, NT // 2):
            block(t, t, 0, NT, col_sp, r_sp, NT // 2)
        for t in range(TT // 2):
            block(8 + t, t, N, TT, col_tm, r_tm, TT // 2,
                  split=(t == TT // 2 - 1))


def build_nc(num_devices=8):
    nc = bacc.Bacc(
        "TRN2",
        target_bir_lowering=False,
        debug=False,
        enable_asserts=True,
        num_devices=num_devices,
    )
    sp = nc.dram_tensor("spatial_nodes", (N, D), F32, kind="ExternalInput").ap()
    tm = nc.dram_tensor("temporal_nodes", (T, D), F32, kind="ExternalInput").ap()
    wp = nc.dram_tensor("wpack", (260,), F32, kind="ExternalInput").ap()
    adj = nc.dram_tensor("adj", (W, W), F16, kind="ExternalOutput").ap()

    with tile.TileContext(nc) as tc:
        _emit(tc, sp, tm, wp, adj)
    nc.compile()
    return nc


def make_in_maps(inputs):
    wpack = np.concatenate(
        [np.asarray(inputs[f"w_{nm}"], np.float32).reshape(-1) for nm in QUADS]
        + [np.asarray(inputs[f"b_{nm}"], np.float32).reshape(-1) for nm in QUADS]
    )
    in_maps = []
    for b in range(B):
        m = {
            "spatial_nodes": np.ascontiguousarray(inputs["spatial_nodes"][b], np.float32),
            "temporal_nodes": np.ascontiguousarray(inputs["temporal_nodes"][b], np.float32),
            "wpack": wpack,
        }
        in_maps.append(m)
    return in_maps


_NC = {}


def run(inputs, trace=False, trace_cores=None):
    if 8 not in _NC:
        _NC[8] = build_nc(8)
    res = run_bass_kernel_spmd(
        _NC[8], make_in_maps(inputs), core_ids=list(range(B)), trace=trace,
        trace_cores=trace_cores,
    )
    out = np.stack(
        [res.results[i]["adj"].astype(np.float32) for i in range(B)], axis=0
    )
    return out, res


def kernel(**inputs) -> np.ndarray:
    out, _ = run(inputs, trace=False)
    return out


# revision 50
# speedup vs baseline: 1.0122x; 1.0122x over previous
"""Trainium2 Bass kernel for the MLPConstructor2 adjacency problem.

Computes, per batch b (one NeuronCore each, 8-way data parallel over B):
    adj[i, j] = tanh(relu(x1_i @ w1 + x2_j @ w2 + b))
for the four (spatial/temporal) quadrants of a (2560, 2560) output.

v8 design (ACT-bound, ~46us ScalarE floor):
- Output is stored as fp16 (tolerance is 2e-2; fp16 adds ~1e-3), halving
  the HBM store traffic to 13.1 MB/core. tanh(relu(x)) == relu(tanh(x)),
  so relu runs first (fused on VectorE) and tanh's fp16 result stores
  directly.
- Column vectors are built fully ON-CHIP (the DRAM scratch round-trip
  costs ~6us/hop in SDMA completion latency): VectorE 32x32-block
  transposes put x^T and w^T on partitions, then 4 rank-32 TensorE
  matmuls per half compute s2 = w2 . x_j AND replicate it across all
  128 partitions into PSUM, with the rhs free dims restrided so PSUM
  column order equals j; one VectorE copy lands each half in SBUF fp16.
- Row scalars: one fused tensor_tensor_reduce per quadrant from the
  same (p t) staging (slot t = row p*nt + t), quadrant biases folded in.
- Output blocks are row-strided to match the (p t) slot layout: block t
  covers rows {16p+t} u {16p+t+8}; per block 4 fused VectorE
  tensor_scalar (add row scalar, max 0) fill an fp16 tmp [128, 5120],
  ScalarE runs ONE tanh (fp16 in/out), one 1.31 MB store writes the 256
  strided rows. First and last blocks are split for a faster ramp/tail.
- All weights+biases arrive pre-packed in one "wpack" input (host-side
  concat), one broadcast DMA. Dummy tanh + 1x1 matmul at t=0 pull the
  ACT table load and PE cold-start off the critical path.
"""

import numpy as np
from contextlib import ExitStack

import concourse.bacc as bacc
import concourse.mybir as mybir
import concourse.tile as tile
from concourse.bass_utils import run_bass_kernel_spmd

B, N, T, D = 8, 2048, 512, 32
W = N + T                     # 2560
NT, TT = N // 128, T // 128   # 16, 4 stat slots per partition
F32 = mybir.dt.float32
F16 = mybir.dt.float16
QUADS = ("ss", "st", "ts", "tt")


def _emit(tc, sp, tm, wp_in, adj):
    nc = tc.nc
    AF = mybir.ActivationFunctionType
    OP = mybir.AluOpType
    with ExitStack() as ctx:
        ctx.enter_context(nc.allow_low_precision(
            reason="fp16 intermediates; tolerance is 2e-2, fp16 adds ~1e-3"
        ))
        const = ctx.enter_context(tc.tile_pool(name="const", bufs=1))
        tmpp = ctx.enter_context(tc.tile_pool(name="tmpp", bufs=2))
        outp = ctx.enter_context(tc.tile_pool(name="outp", bufs=3))
        psum = ctx.enter_context(tc.tile_pool(name="psum", bufs=1, space="PSUM"))

        # ---- one broadcast load for all weights + biases ------------------
        # wpack = [w_ss | w_st | w_ts | w_tt | b_ss b_st b_ts b_tt] (260 f32)
        wp = const.tile([128, 260], F32)
        nc.scalar.dma_start(wp[:], wp_in.unsqueeze(0).broadcast_to((128, 260)))

        def w_row(q):  # first half of w_q: row-side weights
            return wp[:, 64 * q : 64 * q + D]

        def b_q(q):
            return wp[:, 256 + q : 257 + q]

        # ---- warmups: ACT table load + PE cold-start at t=0 ---------------
        warm = const.tile([128, 1], F32)
        nc.vector.memset(warm[:], 0.0)
        nc.scalar.activation(warm[:], warm[:], AF.Tanh)

        # ---- stage inputs, (p t) layout: row p*nt+t at [p, t*D:(t+1)*D] ---
        x_tm = const.tile([128, TT * D], F32)
        nc.sync.dma_start(x_tm[:], tm.rearrange("(p t) d -> p t d", p=128))
        x_sp = const.tile([128, NT * D], F32)
        nc.sync.dma_start(x_sp[:], sp.rearrange("(p t) d -> p t d", p=128))
        # row-rotated copies (partition p <- row (64+p)*nt+t, p<64): the PE
        # column builder needs every 32-row group at partition base 0/32 --
        # base 96 (and explicit tile_position) is rejected/crashes.
        x_tm_sh = const.tile([64, TT * D], F32)
        nc.scalar.dma_start(
            x_tm_sh[:], tm.rearrange("(p t) d -> p t d", p=128)[64:128]
        )
        x_sp_sh = const.tile([64, NT * D], F32)
        nc.scalar.dma_start(
            x_sp_sh[:], sp.rearrange("(p t) d -> p t d", p=128)[64:128]
        )

        # ---- transposed operands for the PE column builder ----------------
        # wT[32a+d, j] = w_col_q(d): 32x32-block transpose of the (identical
        # across partitions) wp columns; then materialized to [32, 128] rows.
        wT = const.tile([128, 4 * D], F32)
        for q in range(4):
            nc.vector.transpose(
                wT[:, 32 * q : 32 * (q + 1)], wp[:, 64 * q + D : 64 * q + 2 * D]
            )
        ones = const.tile([128, 128], F32)
        nc.vector.memset(ones[:], 1.0)
        wmat = const.tile([128, 4 * 128], F32)   # q at [128q : 128(q+1)]
        for q in range(4):
            nc.vector.tensor_scalar(
                wmat[:, 128 * q : 128 * (q + 1)], ones[:],
                wT[:, 32 * q : 32 * q + 1], None, OP.mult,
            )
        # xT[32a+d, 32t+p'] = x[(32a+p')*nt + t, d]; lo = groups 0,1 and
        # hi = groups 2,3 (from the rotated staging), all at bases 0/32.
        xT_sp = const.tile([64, NT * D], F32, name="xT_sp", tag="xT_sp")
        nc.vector.transpose(xT_sp[:], x_sp[0:64, :])
        xT_sph = const.tile([64, NT * D], F32, name="xT_sph", tag="xT_sph")
        nc.vector.transpose(xT_sph[:], x_sp_sh[:])
        xT_tm = const.tile([64, TT * D], F32, name="xT_tm", tag="xT_tm")
        nc.vector.transpose(xT_tm[:], x_tm[0:64, :])
        xT_tmh = const.tile([64, TT * D], F32, name="xT_tmh", tag="xT_tmh")
        nc.vector.transpose(xT_tmh[:], x_tm_sh[:])

        # col[q, j] = w_col . x_j + (bias folded into row stats):
        # per 32-row group a, psum[:, ca + p'*nt + t] = sum_d wmat[d, :] *
        # xT[32a+d, (p', t)] -- column order == j, so one contiguous copy.
        # psum column f = 512a + 32t + p'; output col j = (32a+p')*nt + t,
        # so the psum->SBUF copy un-interleaves via its (strided) input AP.
        def col_half(q_sp, q_tm, name, copies=None):
            # copies: ScalarE idles during the ramp and sits closer to PSUM,
            # so col_sp's un-interleave copies run there; col_tm's stay on
            # VectorE (ScalarE is the steady-state bottleneck).
            def ccopy(dst, src):
                if copies is None:
                    nc.vector.tensor_copy(dst, src)
                else:
                    copies.activation(dst, src, AF.Copy)

            col = const.tile([128, W], F16, name=f"col_{name}", tag=f"col_{name}")
            psn = psum.tile([128, N], F32, name=f"psn_{name}", tag="psn")
            for a in range(4):
                xs = (xT_sp, xT_sph)[a // 2]
                b = 32 * (a % 2)
                nc.tensor.matmul(
                    psn[:, 512 * a : 512 * (a + 1)],
                    wmat[b : b + 32, 128 * q_sp : 128 * (q_sp + 1)],
                    xs[b : b + 32, :],
                )
            for c in range(2):   # copy group-pairs as their matmuls finish
                ccopy(
                    col[:, 1024 * c : 1024 * (c + 1)].rearrange(
                        "q (a p t) -> q a p t", a=2, t=NT
                    ),
                    psn[:, 1024 * c : 1024 * (c + 1)].rearrange(
                        "q (a t p) -> q a p t", a=2, t=NT
                    ),
                )
            # T-part groups each get their own PSUM bank: matmul outputs
            # packed at sub-bank offsets crash the device at runtime.
            pst = psum.tile([128, N], F32, name=f"pst_{name}", tag="pst")
            for a in range(4):
                xs = (xT_tm, xT_tmh)[a // 2]
                b = 32 * (a % 2)
                nc.tensor.matmul(
                    pst[:, 512 * a : 512 * a + 128],
                    wmat[b : b + 32, 128 * q_tm : 128 * (q_tm + 1)],
                    xs[b : b + 32, :],
                )
            ccopy(
                col[:, N:W].rearrange("q (a p t) -> q a p t", a=4, t=TT),
                pst[:].rearrange("q (a t p) -> q a t p", a=4, t=NT)[:, :, 0:TT, :]
                .rearrange("q a t p -> q a p t"),
            )
            return col

        # row stats (slot t = row p*nt + t), quadrant biases folded in.
        def rstat(x, nt, q, dst, name):
            prod = const.tile([128, nt * D], F32, name=f"prod_{name}", tag="prod")
            x3 = x[:].rearrange("p (t d) -> p t d", t=nt)
            p3 = prod[:].rearrange("p (t d) -> p t d", t=nt)
            w3 = w_row(q).unsqueeze(1).broadcast_to((128, nt, D))
            nc.vector.tensor_tensor(p3, x3, w3, OP.mult)
            nc.vector.tensor_reduce(dst, p3, axis=mybir.AxisListType.X, op=OP.add)
            nc.vector.tensor_scalar_add(dst, dst, b_q(q))

        col_sp = col_half(0, 1, "sp", copies=nc.scalar)    # w_ss2, w_st2
        r_sp = const.tile([128, 2 * NT], F32)
        rstat(x_sp, NT, 0, r_sp[:, 0:NT], "r_ss")
        rstat(x_sp, NT, 1, r_sp[:, NT:], "r_st")

        # ---- main loop: strided 256-row blocks -----------------------------
        # spatial block t (t=0..7): rows {16p+t} (h=0) and {16p+t+8} (h=1)
        # temporal block t (t=0..1): rows 2048 + {4p+t} and 2048 + {4p+t+2}
        def block(k, t, base, nt, col, rst, hs, split=False):
            tmp = tmpp.tile([128, 2 * W], F16, name=f"tmp{k}", tag="tmp")
            ot = outp.tile([128, 2 * W], F16, name=f"ot{k}", tag="ot")
            quad = adj[base : base + 128 * nt, :]
            for h in range(2):
                o = h * W
                s = t + h * hs
                rn = rst[:, s : s + 1]
                rt = rst[:, nt + s : nt + s + 1]
                nc.vector.tensor_scalar(
                    tmp[:, o + N : o + W], col[:, N:W], rt, 0.0, OP.add, OP.max
                )
                nc.vector.tensor_scalar(
                    tmp[:, o : o + N], col[:, 0:N], rn, 0.0, OP.add, OP.max
                )
                if split:
                    nc.scalar.activation(ot[:, o : o + W], tmp[:, o : o + W], AF.Tanh)
                    nc.sync.dma_start(
                        quad.rearrange("(p r) w -> p r w", p=128)[:, s : s + 1, :],
                        ot[:, o : o + W].rearrange("p (r w) -> p r w", r=1),
                    )
            if not split:
                nc.scalar.activation(ot[:], tmp[:], AF.Tanh)
                # partition p -> rows base + nt*p + t and base + nt*p + t + hs
                nc.sync.dma_start(
                    quad.rearrange("(p g r) w -> p g r w", p=128, g=2)[
                        :, :, t : t + 1, :
                    ],
                    ot[:].rearrange("p (g w) -> p g w", g=2).unsqueeze(2),
                )

        block(0, 0, 0, NT, col_sp, r_sp, NT // 2, split=True)

        # temporal-column stats + blocks emitted after block 0 so their
        # on-chip col build (PE + VectorE copies) stays off block 0's path.
        col_tm = col_half(2, 3, "tm")    # w_ts2, w_tt2
        r_tm = const.tile([128, 2 * TT], F32)
        rstat(x_tm, TT, 2, r_tm[:, 0:TT], "r_ts")
        rstat(x_tm, TT, 3, r_tm[:, TT:], "r_tt")

        for t in range(1, NT // 2):
            block(t, t, 0, NT, col_sp, r_sp, NT // 2)
        for t in range(TT // 2):
            block(8 + t, t, N, TT, col_tm, r_tm, TT // 2,
                  split=(t == TT // 2 - 1))


def build_nc(num_devices=8):
    nc = bacc.Bacc(
        "TRN2",
        target_bir_lowering=False,
        debug=False,
        enable_asserts=True,
        num_devices=num_devices,
    )
    sp = nc.dram_tensor("spatial_nodes", (N, D), F32, kind="ExternalInput").ap()
    tm = nc.dram_tensor("temporal_nodes", (T, D), F32, kind="ExternalInput").ap()
    wp = nc.dram_tensor("wpack", (260,), F32, kind="ExternalInput").ap()
    adj = nc.dram_tensor("adj", (W, W), F16, kind="ExternalOutput").ap()

    with tile.TileContext(nc) as tc:
        _emit(tc, sp, tm, wp, adj)
    nc.compile()
    return nc


def make_in_maps(inputs):
    wpack = np.concatenate(
        [np.asarray(inputs[f"w_{nm}"], np.float32).reshape(-1) for nm in QUADS]
        + [np.asarray(inputs[f"b_{nm}"], np.float32).reshape(-1) for nm in QUADS]
    )
    in_maps = []
    for b in range(B):
        m = {
            "spatial_nodes": np.ascontiguousarray(inputs["spatial_nodes"][b], np.float32),
            "temporal_nodes": np.ascontiguousarray(inputs["temporal_nodes"][b], np.float32),
            "wpack": wpack,
        }
        in_maps.append(m)
    return in_maps


_NC = {}


def run(inputs, trace=False, trace_cores=None):
    if 8 not in _NC:
        _NC[8] = build_nc(8)
    res = run_bass_kernel_spmd(
        _NC[8], make_in_maps(inputs), core_ids=list(range(B)), trace=trace,
        trace_cores=trace_cores,
    )
    out = np.stack(
        [res.results[i]["adj"].astype(np.float32) for i in range(B)], axis=0
    )
    return out, res


def kernel(**inputs) -> np.ndarray:
    out, _ = run(inputs, trace=False)
    return out
